# revision 11
# baseline (speedup 1.0000x reference)
"""Trainium2 Bass kernel for BasicAttention.

reference math (fp32):
  xf = x.reshape(b, din, hw)               # b=4, din=256, hw=4096
  Q = q_w @ xf   [b, 64, hw]
  K = k_w @ xf   [b, 64, hw]
  V = v_w @ xf   [b, 256, hw]
  S = Q^T K      [b, hw, hw]
  A = softmax(S, axis=-1)
  z = (A @ V^T)^T -> [b, 256, h, w]

Sharding: 8 cores = (batch b in 0..4) x (query half in 0..2). Each core gets
its batch's full xf with columns rotated so its 2048 queries come first
(attention is permutation-invariant over keys, so K/V built from the rotated
xf give identical outputs).

Mixed precision (validated vs reference: rel err ~6e-3, budget 2e-2):
  - x, weights, Q, K in fp16 (e5m10): near-f32r accuracy for the softmax
    exponent, but 16-bit operands enable Fast Weight Load (4x faster
    LDWEIGHTS, otherwise exposed between back-to-back matmuls) and halve
    the x DMA. bf16 here fails (2.5e-2): exp amplifies 8-bit-mantissa
    S error.
  - exp output E and V (vt tiles) are bf16: quantization after the exp is
    benign; E needs bf16's e8 range (values up to e^60, fp16 overflows).

Per-core dataflow:
  - S via 2x PE row tiling: dk=64 only fills half the 128-row array, so two
    independent key-block matmuls run concurrently on row groups 0-63 /
    64-127. k_sb keeps even key-blocks in partitions 0-63 and odd blocks in
    64-127 (no duplication); q_sb holds Q duplicated in both halves. Each S
    pair lands in one [128,1024] 2-bank psum tile.
  - One exp per pair ([128,1024] ACTIVATE straight out of PSUM -> bf16 E),
    amortizing the ~350-cycle ACT instruction overhead.
  - Z matmuls: vt (bf16) as weights, E slices as moving operand; psum
    accumulation over all 32 key blocks.
  - Softmax denominators: DVE adds E's two chunk-halves in bf16 (2x mode),
    accumulates the pair-sums in fp32 across the ptile; then 4 transposed
    ones-matmuls put per-query sums directly on 128 psum partitions [128,4],
    reciprocal on 128 DVE lanes, a DRAM bounce reshapes to [1,512], and a
    rank-1 matmul (ones column x recip row) broadcasts 1/d across the dv
    partitions -- replacing the slow 128-packet partition-broadcast DMA.
    Final normalize fused with the PSUM->SBUF eviction of Z.
"""

import sys
import os

sys.path.insert(0, "/opt/trn_rl_repo")

import numpy as np

B, DIN, H, W = 4, 256, 64, 64
HW = H * W            # 4096 keys
DK, DV = 64, 256
PQ = HW // 2          # 2048 queries per core
PT = 512              # query tile (psum free dim)
QC = 128              # key block (contraction tile)
NPT = PQ // PT        # 4
NQC = HW // QC        # 32 key blocks
NPAIR = NQC // 2      # 16 S pairs per ptile
N_CORES = 8

_cache = {}


def _build():
    if "nc" in _cache:
        return _cache["nc"]

    from contextlib import ExitStack
    import concourse.tile as tile
    from concourse import bacc, mybir

    f32 = mybir.dt.float32
    f32r = mybir.dt.float32r
    f16 = mybir.dt.float16
    bf16 = mybir.dt.bfloat16

    nc = bacc.Bacc("TRN2", target_bir_lowering=False, debug=False,
                   num_devices=N_CORES)

    xb = nc.dram_tensor("xb", [DIN, HW], f16, kind="ExternalInput").ap()
    qwT = nc.dram_tensor("qwT", [DIN, DK], f16, kind="ExternalInput").ap()
    kwT = nc.dram_tensor("kwT", [DIN, DK], f16, kind="ExternalInput").ap()
    vwT = nc.dram_tensor("vwT", [DIN, DV], f16, kind="ExternalInput").ap()
    zout = nc.dram_tensor("zout", [DV, PQ], f32, kind="ExternalOutput").ap()

    with tile.TileContext(nc) as tc, ExitStack() as ctx:
        singles = ctx.enter_context(tc.tile_pool(name="singles", bufs=1))
        vt_pool = ctx.enter_context(tc.tile_pool(name="vt_pool", bufs=NPAIR))
        e_pool = ctx.enter_context(tc.tile_pool(name="e_pool", bufs=5))
        t_pool = ctx.enter_context(tc.tile_pool(name="t_pool", bufs=4))
        sum_pool = ctx.enter_context(tc.tile_pool(name="sum_pool", bufs=2))
        out_pool = ctx.enter_context(tc.tile_pool(name="out_pool", bufs=4))
        dram_pool = ctx.enter_context(tc.tile_pool(name="dram_pool", bufs=2,
                                                   space="DRAM"))
        ps_s = ctx.enter_context(tc.tile_pool(name="ps_s", bufs=2,
                                              space="PSUM"))
        ps_z = ctx.enter_context(tc.tile_pool(name="ps_z", bufs=4, space="PSUM"))

        # ---- weights via SWDGE (parallel with the big x loads below) ----
        w_q0 = singles.tile([128, DK], f16)
        w_q1 = singles.tile([128, DK], f16)
        w_k0 = singles.tile([128, DK], f16)
        w_k1 = singles.tile([128, DK], f16)
        w_v0 = singles.tile([128, DV], f16)
        w_v1 = singles.tile([128, DV], f16)
        nc.sync.dma_start(out=w_k0, in_=kwT[0:128, :])
        nc.sync.dma_start(out=w_k1, in_=kwT[128:256, :])
        nc.scalar.dma_start(out=w_q0, in_=qwT[0:128, :])
        nc.scalar.dma_start(out=w_q1, in_=qwT[128:256, :])
        nc.scalar.dma_start(out=w_v0, in_=vwT[0:128, :])
        nc.scalar.dma_start(out=w_v1, in_=vwT[128:256, :])

        ones_f = singles.tile([128, 1], f32)
        nc.vector.memset(ones_f, 1.0)
        ones_r = singles.tile([1, 128], f32)    # ones row (broadcast lhsT)
        nc.vector.memset(ones_r, 1.0)

        # q_sb: Q duplicated in partition halves (row tiles each stream their
        # own SBUF partition range). k_sb: even key blocks in rows 0-63, odd
        # in rows 64-127 -- kslot i holds blocks (2i, 2i+1).
        q_sb = singles.tile([128, PQ], f16)
        k_sb = singles.tile([128, HW // 2], f16)
        xf0 = singles.tile([128, HW], f16)
        xf1 = singles.tile([128, HW], f16)

        # ---- x load: 512-col blocks round-robined over 4 DMA queues ----
        CHW = 1024                       # proj chunk granularity
        qs = (nc.sync, nc.scalar, nc.gpsimd)
        for j in range(HW // PT):
            sl = slice(j * PT, (j + 1) * PT)
            qs[(2 * j) % 3].dma_start(out=xf0[:, sl], in_=xb[0:128, sl])
            qs[(2 * j + 1) % 3].dma_start(out=xf1[:, sl], in_=xb[128:256, sl])

        # Projections for chunk g (x cols [g*1024,(g+1)*1024) = key blocks
        # 8g..8g+7 = kslots 4g..4g+3). Emitted lazily so chunk g's matmuls
        # interleave with main-loop iterations on earlier chunks.
        vt = [None] * NPAIR   # vt[i]: [128, 512] bf16, blocks (2i, 2i+1)

        def q_proj(i):
            pq = ps_s.tile([DK, PT], f32, name=f"ps_q{i}", tag="ps_s")
            nc.tensor.matmul(pq, w_q0, xf0[:, i * PT:(i + 1) * PT],
                             start=True, stop=False)
            nc.tensor.matmul(pq, w_q1, xf1[:, i * PT:(i + 1) * PT],
                             start=False, stop=True)
            nc.scalar.copy(q_sb[0:64, i * PT:(i + 1) * PT], pq)
            nc.sync.dma_start(out=q_sb[64:128, i * PT:(i + 1) * PT],
                              in_=q_sb[0:64, i * PT:(i + 1) * PT])

        def proj_chunk(g):
            # K: one [64,1024] psum pair; strided eviction splits even/odd
            # key blocks into k_sb partition halves.
            pk = ps_s.tile([DK, 2 * PT], f32, name=f"ps_k{g}", tag="ps_s")
            for hh in range(2):
                sl = slice(g * CHW + hh * PT, g * CHW + (hh + 1) * PT)
                nc.tensor.matmul(pk[:, hh * PT:(hh + 1) * PT], w_k0,
                                 xf0[:, sl], start=True, stop=False)
                nc.tensor.matmul(pk[:, hh * PT:(hh + 1) * PT], w_k1,
                                 xf1[:, sl], start=False, stop=True)
            pk_s = pk.rearrange("p (a b c) -> p b a c", a=4, b=2, c=QC)
            dst = k_sb[:, g * PT:(g + 1) * PT].rearrange(
                "p (a c) -> p a c", a=4, c=QC)
            nc.scalar.copy(dst[0:64], pk_s[:, 0])
            nc.scalar.copy(dst[64:128], pk_s[:, 1])
            if g < 2:
                q_proj(g)
            # V: [128,512] psum pair per vt tile (2 key blocks)
            for c2 in range(g * 4, (g + 1) * 4):
                pv = ps_s.tile([QC, 2 * DV], f32, name=f"ps_v{c2}",
                               tag="ps_s")
                for h in range(2):
                    blk = 2 * c2 + h
                    nc.tensor.matmul(pv[:, h * DV:(h + 1) * DV],
                                     xf0[:, blk * QC:(blk + 1) * QC], w_v0,
                                     start=True, stop=False)
                    nc.tensor.matmul(pv[:, h * DV:(h + 1) * DV],
                                     xf1[:, blk * QC:(blk + 1) * QC], w_v1,
                                     start=False, stop=True)
                vtp = vt_pool.tile([QC, 2 * DV], bf16, name=f"vt{c2}",
                                   tag="vt")
                if c2 % 2 == 0:
                    nc.scalar.copy(vtp, pv)
                else:
                    nc.vector.tensor_copy(vtp, pv)
                vt[c2] = vtp

        proj_chunk(0)

        # ---- attention main loop ----
        deferred = None
        for pt in range(NPT):
            qs_lo = q_sb[0:64, pt * PT:(pt + 1) * PT]
            qs_hi = q_sb[64:128, pt * PT:(pt + 1) * PT]
            pz0 = ps_z.tile([128, PT], f32, name=f"pz0_{pt}", tag="pz")
            pz1 = ps_z.tile([128, PT], f32, name=f"pz1_{pt}", tag="pz")
            acc0 = sum_pool.tile([QC, PT], f32, name=f"acc0_{pt}", tag="acc0")
            acc1 = sum_pool.tile([QC, PT], f32, name=f"acc1_{pt}", tag="acc1")

            def s_pair(i, qs_lo=qs_lo, qs_hi=qs_hi, pt=pt):
                # Row-tiled: block 2i on rows 0-63 -> bank A, block 2i+1 on
                # rows 64-127 -> bank B of the same 2-bank tile.
                ps = ps_s.tile([QC, 2 * PT], f32, name=f"ps_{pt}_{i}",
                               tag="ps_s")
                nc.tensor.matmul(ps[:, 0:PT], k_sb[0:64, i * QC:(i + 1) * QC],
                                 qs_lo, start=True, stop=True,
                                 tile_position=(0, 0))
                nc.tensor.matmul(ps[:, PT:2 * PT],
                                 k_sb[64:128, i * QC:(i + 1) * QC],
                                 qs_hi, start=True, stop=True,
                                 tile_position=(64, 0))
                return ps

            pend = [s_pair(0), s_pair(1)]

            for i in range(NPAIR):
                # exp of both blocks in one ACTIVATE (2 psum banks)
                e = e_pool.tile([QC, 2 * PT], bf16, name=f"e_{pt}_{i}",
                                tag="e")
                nc.scalar.activation(e, pend.pop(0),
                                     func=mybir.ActivationFunctionType.Exp)
                if pt == 0 and i in (1, 5, 9):
                    proj_chunk(i // 4 + 1)
                if pt == 1 and i == 1:
                    q_proj(2)
                    q_proj(3)
                # Z matmuls: blocks 2i (E cols 0:512) and 2i+1 (cols 512:1024)
                for v, pz in ((0, pz0), (1, pz1)):
                    hs = (1, 0) if v == 0 else (0, 1)
                    for idx, h in enumerate(hs):
                        nc.tensor.matmul(
                            pz,
                            vt[i][:, h * DV + v * 128:h * DV + (v + 1) * 128],
                            e[:, h * PT:(h + 1) * PT],
                            start=(i == 0 and idx == 0),
                            stop=(i == NPAIR - 1 and idx == 1))
                if i + 2 < NPAIR:
                    pend.append(s_pair(i + 2))
                # softmax key-sums: bf16 pair add (2x DVE), fp32 accumulate
                t = t_pool.tile([QC, PT], bf16, name=f"t_{pt}_{i}", tag="t")
                nc.vector.tensor_add(t, e[:, 0:PT], e[:, PT:2 * PT])
                acc = acc0 if i % 2 == 0 else acc1
                if i < 2:
                    nc.vector.tensor_copy(acc, t)
                else:
                    nc.vector.tensor_add(acc, acc, t)
                if i == 2 and deferred is not None:
                    deferred()
                    deferred = None

            def make_tail(pt=pt, acc0=acc0, acc1=acc1, pz0=pz0, pz1=pz1):
                def tail():
                    # fold acc halves, permuting queries q=4j+s to col
                    # s*128+j so the transposed sums-matmuls land d in the
                    # same [128,4] layout the DRAM bounce expects.
                    accr = sum_pool.tile([QC, PT], f32,
                                         name=f"accr{pt}", tag="accr")
                    nc.vector.tensor_add(
                        accr.rearrange("p (s j) -> p s j", s=4, j=128),
                        acc0.rearrange("p (j s) -> p s j", j=128, s=4),
                        acc1.rearrange("p (j s) -> p s j", j=128, s=4))
                    # transposed sums: d[q] on 128 psum partitions [128,4]
                    # (bank A); rank-1 broadcast matmul output in bank B.
                    tps = ps_s.tile([128, 2 * PT], f32,
                                    name=f"tps{pt}", tag="ps_s")
                    d_ps = tps[:, 0:4]
                    for s in range(4):
                        nc.tensor.matmul(d_ps[:, s:s + 1],
                                         accr[:, s * 128:(s + 1) * 128],
                                         ones_f,
                                         start=(s == 0), stop=(s == 3),
                                         skip_group_check=True)
                    rd = sum_pool.tile([128, 4], f32,
                                       name=f"rd{pt}", tag="rd")
                    nc.vector.reciprocal(rd, d_ps)
                    # reshape [128,4] -> [1,512] via DRAM bounce
                    rscr = dram_pool.tile([1, PT], f32,
                                          name=f"rscr{pt}", tag="rscr")
                    # rd[p, f] holds 1/d for query 4p+f (via the fold
                    # permute) -> contiguous 16B-per-partition DMA
                    nc.sync.dma_start(
                        out=rscr.rearrange("o (p f) -> (o p) f", p=128),
                        in_=rd)
                    d_row = sum_pool.tile([1, PT], f32,
                                          name=f"drow{pt}", tag="drow")
                    nc.sync.dma_start(out=d_row, in_=rscr)
                    bc_ps = tps[:, PT:2 * PT]
                    nc.tensor.matmul(bc_ps, ones_r, d_row,
                                     start=True, stop=True,
                                     skip_group_check=True)
                    bc = sum_pool.tile([128, PT], f32,
                                       name=f"bc{pt}", tag="bc")
                    nc.scalar.copy(bc, bc_ps)
                    out0 = out_pool.tile([128, PT], f32,
                                         name=f"out0_{pt}", tag="out")
                    out1 = out_pool.tile([128, PT], f32,
                                         name=f"out1_{pt}", tag="out")
                    nc.vector.tensor_mul(out0, pz0, bc)
                    nc.vector.tensor_mul(out1, pz1, bc)
                    nc.sync.dma_start(out=zout[0:128, pt * PT:(pt + 1) * PT],
                                      in_=out0)
                    nc.sync.dma_start(
                        out=zout[128:256, pt * PT:(pt + 1) * PT], in_=out1)
                return tail

            deferred = make_tail()
        deferred()

    nc.compile()
    _cache["nc"] = nc
    return nc


def _in_maps(x, q_w, k_w, v_w):
    xf = np.asarray(x, np.float32).reshape(B, DIN, HW)
    qwT = np.ascontiguousarray(np.asarray(q_w, np.float32).T.astype(np.float16))
    kwT = np.ascontiguousarray(np.asarray(k_w, np.float32).T.astype(np.float16))
    vwT = np.ascontiguousarray(np.asarray(v_w, np.float32).T.astype(np.float16))
    maps = []
    for c in range(N_CORES):
        b, half = divmod(c, 2)
        xbc = xf[b] if half == 0 else np.roll(xf[b], -PQ, axis=1)
        maps.append({"xb": np.ascontiguousarray(xbc.astype(np.float16)),
                     "qwT": qwT, "kwT": kwT, "vwT": vwT})
    return maps


def _gather(results):
    z = np.empty((B, DV, HW), np.float32)
    for c in range(N_CORES):
        b, half = divmod(c, 2)
        z[b][:, half * PQ:(half + 1) * PQ] = results[c]["zout"]
    return z.reshape(B, DV, H, W)


def _run(x, q_w, k_w, v_w, trace=False):
    from concourse import bass_utils
    nc = _build()
    res = bass_utils.run_bass_kernel_spmd(
        nc, _in_maps(x, q_w, k_w, v_w), core_ids=list(range(N_CORES)),
        trace=trace)
    return _gather(res.results), res


def kernel(x, q_w, k_w, v_w):
    z, _ = _run(x, q_w, k_w, v_w)
    return z


# revision 12
# speedup vs baseline: 1.0051x; 1.0051x over previous
"""Trainium2 Bass kernel for BasicAttention.

reference math (fp32):
  xf = x.reshape(b, din, hw)               # b=4, din=256, hw=4096
  Q = q_w @ xf   [b, 64, hw]
  K = k_w @ xf   [b, 64, hw]
  V = v_w @ xf   [b, 256, hw]
  S = Q^T K      [b, hw, hw]
  A = softmax(S, axis=-1)
  z = (A @ V^T)^T -> [b, 256, h, w]

Sharding: 8 cores = (batch b in 0..4) x (query half in 0..2). Each core gets
its batch's full xf with columns rotated so its 2048 queries come first
(attention is permutation-invariant over keys, so K/V built from the rotated
xf give identical outputs).

Mixed precision (validated vs reference: rel err ~6e-3, budget 2e-2):
  - x, weights, Q, K in fp16 (e5m10): near-f32r accuracy for the softmax
    exponent, but 16-bit operands enable Fast Weight Load (4x faster
    LDWEIGHTS, otherwise exposed between back-to-back matmuls) and halve
    the x DMA. bf16 here fails (2.5e-2): exp amplifies 8-bit-mantissa
    S error.
  - exp output E and V (vt tiles) are bf16: quantization after the exp is
    benign; E needs bf16's e8 range (values up to e^60, fp16 overflows).

Per-core dataflow:
  - S via 2x PE row tiling: dk=64 only fills half the 128-row array, so two
    independent key-block matmuls run concurrently on row groups 0-63 /
    64-127. k_sb keeps even key-blocks in partitions 0-63 and odd blocks in
    64-127 (no duplication); q_sb holds Q duplicated in both halves. Each S
    pair lands in one [128,1024] 2-bank psum tile.
  - One exp per pair ([128,1024] ACTIVATE straight out of PSUM -> bf16 E),
    amortizing the ~350-cycle ACT instruction overhead.
  - Z matmuls: vt (bf16) as weights, E slices as moving operand; psum
    accumulation over all 32 key blocks.
  - Softmax denominators: DVE adds E's two chunk-halves in bf16 (2x mode),
    accumulates the pair-sums in fp32 across the ptile; then 4 transposed
    ones-matmuls put per-query sums directly on 128 psum partitions [128,4],
    reciprocal on 128 DVE lanes, a DRAM bounce reshapes to [1,512], and a
    rank-1 matmul (ones column x recip row) broadcasts 1/d across the dv
    partitions -- replacing the slow 128-packet partition-broadcast DMA.
    Final normalize fused with the PSUM->SBUF eviction of Z.
"""

import sys
import os

sys.path.insert(0, "/opt/trn_rl_repo")

import numpy as np

B, DIN, H, W = 4, 256, 64, 64
HW = H * W            # 4096 keys
DK, DV = 64, 256
PQ = HW // 2          # 2048 queries per core
PT = 512              # query tile (psum free dim)
QC = 128              # key block (contraction tile)
NPT = PQ // PT        # 4
NQC = HW // QC        # 32 key blocks
NPAIR = NQC // 2      # 16 S pairs per ptile
N_CORES = 8

_cache = {}


def _build():
    if "nc" in _cache:
        return _cache["nc"]

    from contextlib import ExitStack
    import concourse.tile as tile
    from concourse import bacc, mybir

    f32 = mybir.dt.float32
    f32r = mybir.dt.float32r
    f16 = mybir.dt.float16
    bf16 = mybir.dt.bfloat16

    nc = bacc.Bacc("TRN2", target_bir_lowering=False, debug=False,
                   num_devices=N_CORES)

    xb = nc.dram_tensor("xb", [DIN, HW], f16, kind="ExternalInput").ap()
    qwT = nc.dram_tensor("qwT", [DIN, DK], f16, kind="ExternalInput").ap()
    kwT = nc.dram_tensor("kwT", [DIN, DK], f16, kind="ExternalInput").ap()
    vwT = nc.dram_tensor("vwT", [DIN, DV], f16, kind="ExternalInput").ap()
    zout = nc.dram_tensor("zout", [DV, PQ], f32, kind="ExternalOutput").ap()

    with tile.TileContext(nc) as tc, ExitStack() as ctx:
        singles = ctx.enter_context(tc.tile_pool(name="singles", bufs=1))
        vt_pool = ctx.enter_context(tc.tile_pool(name="vt_pool", bufs=NPAIR))
        e_pool = ctx.enter_context(tc.tile_pool(name="e_pool", bufs=5))
        t_pool = ctx.enter_context(tc.tile_pool(name="t_pool", bufs=4))
        sum_pool = ctx.enter_context(tc.tile_pool(name="sum_pool", bufs=2))
        out_pool = ctx.enter_context(tc.tile_pool(name="out_pool", bufs=4))
        dram_pool = ctx.enter_context(tc.tile_pool(name="dram_pool", bufs=2,
                                                   space="DRAM"))
        ps_s = ctx.enter_context(tc.tile_pool(name="ps_s", bufs=2,
                                              space="PSUM"))
        ps_z = ctx.enter_context(tc.tile_pool(name="ps_z", bufs=4, space="PSUM"))

        # ---- weights via SWDGE (parallel with the big x loads below) ----
        w_q0 = singles.tile([128, DK], f16)
        w_q1 = singles.tile([128, DK], f16)
        w_k0 = singles.tile([128, DK], f16)
        w_k1 = singles.tile([128, DK], f16)
        w_v0 = singles.tile([128, DV], f16)
        w_v1 = singles.tile([128, DV], f16)
        nc.sync.dma_start(out=w_k0, in_=kwT[0:128, :])
        nc.sync.dma_start(out=w_k1, in_=kwT[128:256, :])
        nc.scalar.dma_start(out=w_q0, in_=qwT[0:128, :])
        nc.scalar.dma_start(out=w_q1, in_=qwT[128:256, :])
        nc.scalar.dma_start(out=w_v0, in_=vwT[0:128, :])
        nc.scalar.dma_start(out=w_v1, in_=vwT[128:256, :])

        ones_f = singles.tile([128, 1], f32)
        nc.vector.memset(ones_f, 1.0)
        ones_r = singles.tile([1, 128], f32)    # ones row (broadcast lhsT)
        nc.vector.memset(ones_r, 1.0)

        # q_sb: Q duplicated in partition halves (row tiles each stream their
        # own SBUF partition range). k_sb: even key blocks in rows 0-63, odd
        # in rows 64-127 -- kslot i holds blocks (2i, 2i+1).
        q_sb = singles.tile([128, PQ], f16)
        k_sb = singles.tile([128, HW // 2], f16)
        xf0 = singles.tile([128, HW], f16)
        xf1 = singles.tile([128, HW], f16)

        # ---- x load: 512-col blocks round-robined over 4 DMA queues ----
        CHW = 1024                       # proj chunk granularity
        qs = (nc.sync, nc.scalar, nc.gpsimd)
        for j in range(HW // PT):
            sl = slice(j * PT, (j + 1) * PT)
            qs[(2 * j) % 3].dma_start(out=xf0[:, sl], in_=xb[0:128, sl])
            qs[(2 * j + 1) % 3].dma_start(out=xf1[:, sl], in_=xb[128:256, sl])

        # Projections for chunk g (x cols [g*1024,(g+1)*1024) = key blocks
        # 8g..8g+7 = kslots 4g..4g+3). Emitted lazily so chunk g's matmuls
        # interleave with main-loop iterations on earlier chunks.
        vt = [None] * NPAIR   # vt[i]: [128, 512] bf16, blocks (2i, 2i+1)

        def q_proj(i):
            pq = ps_s.tile([DK, PT], f32, name=f"ps_q{i}", tag="ps_s")
            nc.tensor.matmul(pq, w_q0, xf0[:, i * PT:(i + 1) * PT],
                             start=True, stop=False)
            nc.tensor.matmul(pq, w_q1, xf1[:, i * PT:(i + 1) * PT],
                             start=False, stop=True)
            nc.scalar.copy(q_sb[0:64, i * PT:(i + 1) * PT], pq)
            if i == 0:
                # engine copy: a dup DMA would queue behind the x load and
                # stall ptile 0's first row-tiled S matmul by ~10us
                nc.vector.tensor_copy(q_sb[64:128, i * PT:(i + 1) * PT], pq)
            else:
                nc.sync.dma_start(out=q_sb[64:128, i * PT:(i + 1) * PT],
                                  in_=q_sb[0:64, i * PT:(i + 1) * PT])

        def proj_chunk(g):
            # K: one [64,1024] psum pair; strided eviction splits even/odd
            # key blocks into k_sb partition halves.
            pk = ps_s.tile([DK, 2 * PT], f32, name=f"ps_k{g}", tag="ps_s")
            for hh in range(2):
                sl = slice(g * CHW + hh * PT, g * CHW + (hh + 1) * PT)
                nc.tensor.matmul(pk[:, hh * PT:(hh + 1) * PT], w_k0,
                                 xf0[:, sl], start=True, stop=False)
                nc.tensor.matmul(pk[:, hh * PT:(hh + 1) * PT], w_k1,
                                 xf1[:, sl], start=False, stop=True)
            pk_s = pk.rearrange("p (a b c) -> p b a c", a=4, b=2, c=QC)
            dst = k_sb[:, g * PT:(g + 1) * PT].rearrange(
                "p (a c) -> p a c", a=4, c=QC)
            nc.scalar.copy(dst[0:64], pk_s[:, 0])
            nc.scalar.copy(dst[64:128], pk_s[:, 1])
            if g < 2:
                q_proj(g)
            # V: [128,512] psum pair per vt tile (2 key blocks)
            for c2 in range(g * 4, (g + 1) * 4):
                pv = ps_s.tile([QC, 2 * DV], f32, name=f"ps_v{c2}",
                               tag="ps_s")
                for h in range(2):
                    blk = 2 * c2 + h
                    nc.tensor.matmul(pv[:, h * DV:(h + 1) * DV],
                                     xf0[:, blk * QC:(blk + 1) * QC], w_v0,
                                     start=True, stop=False)
                    nc.tensor.matmul(pv[:, h * DV:(h + 1) * DV],
                                     xf1[:, blk * QC:(blk + 1) * QC], w_v1,
                                     start=False, stop=True)
                vtp = vt_pool.tile([QC, 2 * DV], bf16, name=f"vt{c2}",
                                   tag="vt")
                if c2 % 2 == 0:
                    nc.scalar.copy(vtp, pv)
                else:
                    nc.vector.tensor_copy(vtp, pv)
                vt[c2] = vtp

        proj_chunk(0)

        # ---- attention main loop ----
        deferred = None
        for pt in range(NPT):
            qs_lo = q_sb[0:64, pt * PT:(pt + 1) * PT]
            qs_hi = q_sb[64:128, pt * PT:(pt + 1) * PT]
            pz0 = ps_z.tile([128, PT], f32, name=f"pz0_{pt}", tag="pz")
            pz1 = ps_z.tile([128, PT], f32, name=f"pz1_{pt}", tag="pz")
            acc0 = sum_pool.tile([QC, PT], f32, name=f"acc0_{pt}", tag="acc0")
            acc1 = sum_pool.tile([QC, PT], f32, name=f"acc1_{pt}", tag="acc1")

            def s_pair(i, qs_lo=qs_lo, qs_hi=qs_hi, pt=pt):
                # Row-tiled: block 2i on rows 0-63 -> bank A, block 2i+1 on
                # rows 64-127 -> bank B of the same 2-bank tile.
                ps = ps_s.tile([QC, 2 * PT], f32, name=f"ps_{pt}_{i}",
                               tag="ps_s")
                nc.tensor.matmul(ps[:, 0:PT], k_sb[0:64, i * QC:(i + 1) * QC],
                                 qs_lo, start=True, stop=True,
                                 tile_position=(0, 0))
                nc.tensor.matmul(ps[:, PT:2 * PT],
                                 k_sb[64:128, i * QC:(i + 1) * QC],
                                 qs_hi, start=True, stop=True,
                                 tile_position=(64, 0))
                return ps

            pend = [s_pair(0), s_pair(1)]

            for i in range(NPAIR):
                # exp of both blocks in one ACTIVATE (2 psum banks)
                e = e_pool.tile([QC, 2 * PT], bf16, name=f"e_{pt}_{i}",
                                tag="e")
                nc.scalar.activation(e, pend.pop(0),
                                     func=mybir.ActivationFunctionType.Exp)
                if pt == 0 and i in (1, 5, 9):
                    proj_chunk(i // 4 + 1)
                if pt == 1 and i == 1:
                    q_proj(2)
                    q_proj(3)
                # Z matmuls: blocks 2i (E cols 0:512) and 2i+1 (cols 512:1024)
                for v, pz in ((0, pz0), (1, pz1)):
                    hs = (1, 0) if v == 0 else (0, 1)
                    for idx, h in enumerate(hs):
                        nc.tensor.matmul(
                            pz,
                            vt[i][:, h * DV + v * 128:h * DV + (v + 1) * 128],
                            e[:, h * PT:(h + 1) * PT],
                            start=(i == 0 and idx == 0),
                            stop=(i == NPAIR - 1 and idx == 1))
                if i + 2 < NPAIR:
                    pend.append(s_pair(i + 2))
                # softmax key-sums: bf16 pair add (2x DVE), fp32 accumulate
                t = t_pool.tile([QC, PT], bf16, name=f"t_{pt}_{i}", tag="t")
                nc.vector.tensor_add(t, e[:, 0:PT], e[:, PT:2 * PT])
                acc = acc0 if i % 2 == 0 else acc1
                if i < 2:
                    nc.vector.tensor_copy(acc, t)
                else:
                    nc.vector.tensor_add(acc, acc, t)
                if i == 2 and deferred is not None:
                    deferred()
                    deferred = None

            def make_tail(pt=pt, acc0=acc0, acc1=acc1, pz0=pz0, pz1=pz1):
                def tail():
                    # fold acc halves, permuting queries q=4j+s to col
                    # s*128+j so the transposed sums-matmuls land d in the
                    # same [128,4] layout the DRAM bounce expects.
                    accr = sum_pool.tile([QC, PT], f32,
                                         name=f"accr{pt}", tag="accr")
                    nc.vector.tensor_add(
                        accr.rearrange("p (s j) -> p s j", s=4, j=128),
                        acc0.rearrange("p (j s) -> p s j", j=128, s=4),
                        acc1.rearrange("p (j s) -> p s j", j=128, s=4))
                    # transposed sums: d[q] on 128 psum partitions [128,4]
                    # (bank A); rank-1 broadcast matmul output in bank B.
                    tps = ps_s.tile([128, 2 * PT], f32,
                                    name=f"tps{pt}", tag="ps_s")
                    d_ps = tps[:, 0:4]
                    for s in range(4):
                        nc.tensor.matmul(d_ps[:, s:s + 1],
                                         accr[:, s * 128:(s + 1) * 128],
                                         ones_f,
                                         start=(s == 0), stop=(s == 3),
                                         skip_group_check=True)
                    rd = sum_pool.tile([128, 4], f32,
                                       name=f"rd{pt}", tag="rd")
                    nc.vector.reciprocal(rd, d_ps)
                    # reshape [128,4] -> [1,512] via DRAM bounce
                    rscr = dram_pool.tile([1, PT], f32,
                                          name=f"rscr{pt}", tag="rscr")
                    # rd[p, f] holds 1/d for query 4p+f (via the fold
                    # permute) -> contiguous 16B-per-partition DMA
                    nc.sync.dma_start(
                        out=rscr.rearrange("o (p f) -> (o p) f", p=128),
                        in_=rd)
                    d_row = sum_pool.tile([1, PT], f32,
                                          name=f"drow{pt}", tag="drow")
                    nc.sync.dma_start(out=d_row, in_=rscr)
                    bc_ps = tps[:, PT:2 * PT]
                    nc.tensor.matmul(bc_ps, ones_r, d_row,
                                     start=True, stop=True,
                                     skip_group_check=True)
                    bc = sum_pool.tile([128, PT], f32,
                                       name=f"bc{pt}", tag="bc")
                    nc.scalar.copy(bc, bc_ps)
                    out0 = out_pool.tile([128, PT], f32,
                                         name=f"out0_{pt}", tag="out")
                    out1 = out_pool.tile([128, PT], f32,
                                         name=f"out1_{pt}", tag="out")
                    nc.vector.tensor_mul(out0, pz0, bc)
                    nc.vector.tensor_mul(out1, pz1, bc)
                    nc.sync.dma_start(out=zout[0:128, pt * PT:(pt + 1) * PT],
                                      in_=out0)
                    nc.sync.dma_start(
                        out=zout[128:256, pt * PT:(pt + 1) * PT], in_=out1)
                return tail

            deferred = make_tail()
        deferred()

    nc.compile()
    _cache["nc"] = nc
    return nc


def _in_maps(x, q_w, k_w, v_w):
    xf = np.asarray(x, np.float32).reshape(B, DIN, HW)
    qwT = np.ascontiguousarray(np.asarray(q_w, np.float32).T.astype(np.float16))
    kwT = np.ascontiguousarray(np.asarray(k_w, np.float32).T.astype(np.float16))
    vwT = np.ascontiguousarray(np.asarray(v_w, np.float32).T.astype(np.float16))
    maps = []
    for c in range(N_CORES):
        b, half = divmod(c, 2)
        xbc = xf[b] if half == 0 else np.roll(xf[b], -PQ, axis=1)
        maps.append({"xb": np.ascontiguousarray(xbc.astype(np.float16)),
                     "qwT": qwT, "kwT": kwT, "vwT": vwT})
    return maps


def _gather(results):
    z = np.empty((B, DV, HW), np.float32)
    for c in range(N_CORES):
        b, half = divmod(c, 2)
        z[b][:, half * PQ:(half + 1) * PQ] = results[c]["zout"]
    return z.reshape(B, DV, H, W)


def _run(x, q_w, k_w, v_w, trace=False):
    from concourse import bass_utils
    nc = _build()
    res = bass_utils.run_bass_kernel_spmd(
        nc, _in_maps(x, q_w, k_w, v_w), core_ids=list(range(N_CORES)),
        trace=trace)
    return _gather(res.results), res


def kernel(x, q_w, k_w, v_w):
    z, _ = _run(x, q_w, k_w, v_w)
    return z
